# revision 29
# baseline (speedup 1.0000x reference)
"""Trainium2 Bass kernel for the HCFDA dense-CNN module (bf16 I/O).

Math used (exact reassociations of the reference):
  1. The 256x256 1x1 DCT conv is only consumed through a channel-mean, so
     temp[b,h,w] = sum_c m[c] * x[b,c,h,w]  with  m = dct_w.mean(axis=0).
  2. The 3-step diffusion is a polynomial in commuting 1-D reflect-shift
     operators:  T' = c2*T + Rv(Rh T)  with  Rh = S_l + S_r + 4 I (cols,
     free axis) and Rv = alpha*a*(S_u + S_d) + c1*I (rows, via PE matmul).
     Hence T3 = c2^3*T + sum_n C(3,n) c2^(3-n) Rv^n (Rh^n T), n=1..3,
     with Rv^n staged as three precomputed 128x128 lhsT matrices.
  3. SE branch: pooled stats -> two tiny FCs -> sigmoid, per reference.
  out = x * sigmoid(att[c] * sigmoid(T3)[h,w])

Performance structure (per core; tolerance 2e-2 allows bf16 I/O, which
halves HBM traffic vs f32 — all accumulation stays f32):
  - Pooled stats split across engines: ACT Copy+accum_out sums for early
    chunks plus tree roots; DVE pairwise add/max trees (TensorTensor gets
    the 2x packed mode; TensorReduce does not, so trees beat flat
    reduces) with only two final reduces.
  - GEMV temp runs m-stationary (replicated 8-wide) in bf16 on the PE;
    chunks pack 3-per-PSUM-tile at partition bases {0,32,64}; ACT
    evacuates each pack with one Copy, a reshape DMA lands rows in Tp.
  - The 3-step diffusion is applied in closed form (one DVE shift-chain
    per power of Rh, three accumulated PE matmuls, one STT).
  - Phase B: PE ones-matmul broadcasts heat rows to all partitions; ACT
    fuses sigmoid(att*heat) via per-partition scale (the pacing engine);
    DVE multiplies with resident bf16 x; DMA-out rides underneath.

Sharding: pure data parallel, one batch element per NeuronCore (B=8).
"""

import numpy as np
from contextlib import ExitStack

B, C, H, W = 8, 256, 128, 128
HW = H * W           # 16384
NCHUNK = 8           # x chunks over HW
CH = HW // NCHUNK    # 2048
N_CORES = 8


def _reflect(i, n):
    if i < 0:
        return -i
    if i >= n:
        return 2 * (n - 1) - i
    return i


def _build_program(c1, c2):
    from concourse import bass, mybir, tile
    from concourse import bacc

    f32 = mybir.dt.float32
    bf16 = mybir.dt.bfloat16
    AF = mybir.ActivationFunctionType
    ALU = mybir.AluOpType
    AX = mybir.AxisListType
    c2p3 = c2 * c2 * c2

    nc = bacc.Bacc("TRN2", target_bir_lowering=False, debug=False,
                   num_devices=N_CORES)

    xb = nc.dram_tensor("xb", [C, HW], bf16, kind="ExternalInput").ap()
    mf = nc.dram_tensor("mf", [128, 16], bf16, kind="ExternalInput").ap()
    gmd = [nc.dram_tensor(f"gm{n}", [128, 128], f32,
                          kind="ExternalInput").ap() for n in range(3)]
    w1d = nc.dram_tensor("w1t", [128, 32], f32, kind="ExternalInput").ap()
    w2d = nc.dram_tensor("w2t", [16, 256], f32, kind="ExternalInput").ap()
    b1d = nc.dram_tensor("b1c", [16, 1], f32, kind="ExternalInput").ap()
    b2d = nc.dram_tensor("b2c", [128, 2], f32, kind="ExternalInput").ap()
    ond = nc.dram_tensor("onr", [1, 128], bf16, kind="ExternalInput").ap()
    outd = nc.dram_tensor("out", [C, HW], bf16, kind="ExternalOutput").ap()

    with tile.TileContext(nc) as tc, ExitStack() as ctx:
        const = ctx.enter_context(tc.tile_pool(name="const", bufs=1))
        xpool = ctx.enter_context(tc.tile_pool(name="xp", bufs=1))
        work = ctx.enter_context(tc.tile_pool(name="work", bufs=2))
        stat = ctx.enter_context(tc.tile_pool(name="stat", bufs=1))
        actx = ctx.enter_context(ExitStack())
        psT = actx.enter_context(tc.tile_pool(name="psT", bufs=1, space="PSUM"))
        psD = actx.enter_context(tc.tile_pool(name="psD", bufs=1, space="PSUM"))
        psF = actx.enter_context(tc.tile_pool(name="psF", bufs=2, space="PSUM"))

        m_sb = const.tile([128, 16], bf16, tag="m", name="m")
        nc.sync.dma_start(out=m_sb[:], in_=mf)
        g_sb = []
        for n in range(3):
            g = const.tile([128, 128], f32, tag=f"g{n}", name=f"g{n}")
            nc.sync.dma_start(out=g[:], in_=gmd[n])
            g_sb.append(g)
        w1_sb = const.tile([128, 32], f32, tag="w1", name="w1")
        nc.sync.dma_start(out=w1_sb[:], in_=w1d)
        w2_sb = const.tile([16, 256], f32, tag="w2", name="w2")
        nc.sync.dma_start(out=w2_sb[:], in_=w2d)
        b1_sb = const.tile([16, 1], f32, tag="b1", name="b1")
        nc.sync.dma_start(out=b1_sb[:], in_=b1d)
        b2_sb = const.tile([128, 2], f32, tag="b2", name="b2")
        nc.sync.dma_start(out=b2_sb[:], in_=b2d)
        on_sb = const.tile([1, 128], bf16, tag="onr", name="onr")
        nc.sync.dma_start(out=on_sb[:], in_=ond)
        warm = const.tile([1, 2], f32, tag="warm", name="warm")
        nc.scalar.activation(warm[:], b2_sb[0:1, 0:2], AF.Sigmoid)

        sums = stat.tile([128, 2, 6], f32, tag="sums", name="sums")
        ms = stat.tile([128, 2, 2], f32, tag="ms", name="ms")
        junk = stat.tile([128, CH], bf16, tag="junk", name="junk")
        heat = stat.tile([128, W], bf16, tag="heat", name="heat")
        hbuf = stat.tile([1, HW], bf16, tag="hbuf", name="hbuf")
        Tp = stat.tile([128, W + 2], f32, tag="Tp", name="Tp")
        ymax = stat.tile([128, 2], f32, tag="ymax", name="ymax")
        ysum = stat.tile([128, 2], f32, tag="ysum", name="ysum")
        # pairwise-tree temporaries
        gmx = {(t, i): stat.tile([128, CH], bf16, tag=f"g{t}{i}",
                                 name=f"g{t}{i}")
               for t in range(2) for i in range(3)}
        ta = [stat.tile([128, CH], bf16, tag=f"ta{t}", name=f"ta{t}")
              for t in range(2)]

        # ---------- x loads (SP-issued, one DMA per channel-half chunk;
        # the first chunk is split into halves so compute starts sooner) --
        xt = {}
        for j in range(NCHUNK):
            for t in range(2):
                xt[t, j] = xpool.tile([128, CH], bf16, tag=f"x{t}_{j}",
                                      name=f"x{t}_{j}")
                nsp = 4 if j < 2 else 2
                cw = CH // nsp
                for q in range(nsp):
                    nc.sync.dma_start(
                        out=xt[t, j][:, q * cw:(q + 1) * cw],
                        in_=xb[t * 128:(t + 1) * 128,
                               j * CH + q * cw:j * CH + (q + 1) * cw])

        def xv(t, j):
            return xt[t, j][:]

        # ---------- phase A: GEMV (bf16) + pooled stats -------------------
        pT = [psT.tile([128, 1024], f32, tag=f"psT{i}", name=f"psT{i}")
              for i in range(2)]

        def emit_evac(p, nk):
            S3 = work.tile([128, 1024], f32, tag="s3", name="s3", bufs=2)
            nc.scalar.activation(S3[0:32 * (nk - 1) + 8, :],
                                 pT[p % 2][0:32 * (nk - 1) + 8, :],
                                 AF.Copy)
            sv = S3[:].rearrange("(a b) f -> a b f", a=4, b=32)
            nc.sync.dma_start(out=Tp[24 * p:24 * p + 8 * nk, 1:W + 1],
                              in_=sv[0:nk, 0:1, :])

        def emit_stats(j):
            # sums: flat ACT accums for j<=4, a short DVE tree for 5..7.
            # maxs: running DVE tree arranged so chunks 0..6 are fully
            # reduced before chunk 7 lands; only its own reduce trails.
            for t in range(2):
                if j <= 4:
                    nc.scalar.activation(junk[:], xv(t, j), AF.Copy,
                                         accum_out=sums[:, t, j:j + 1])
                if j in (1, 3, 5):
                    i = (j - 1) // 2
                    nc.vector.tensor_max(gmx[t, i][:], xv(t, j - 1),
                                         xv(t, j))
                if j == 3:
                    nc.vector.tensor_max(gmx[t, 0][:], gmx[t, 0][:],
                                         gmx[t, 1][:])
                if j == 6:
                    nc.vector.tensor_max(gmx[t, 2][:], gmx[t, 2][:],
                                         xv(t, 6))
                    nc.vector.tensor_max(gmx[t, 0][:], gmx[t, 0][:],
                                         gmx[t, 2][:])
                    nc.vector.reduce_max(ms[:, t, 0:1], gmx[t, 0][:],
                                         axis=AX.X)
                    nc.vector.tensor_add(ta[t][:], xv(t, 5), xv(t, 6))
                if j == 7:
                    nc.vector.reduce_max(ms[:, t, 1:2], xv(t, 7), axis=AX.X)
                    nc.vector.tensor_max(ymax[:, t:t + 1], ms[:, t, 0:1],
                                         ms[:, t, 1:2])
                    nc.vector.tensor_add(ta[t][:], ta[t][:], xv(t, 7))
                    nc.scalar.activation(junk[:], ta[t][:], AF.Copy,
                                         accum_out=sums[:, t, 5:6])

        for k in range(16):   # temp chunks of 1024 cols
            p, r = divmod(k, 3)
            if r == 0 and p >= 2:
                emit_evac(p - 2, 3)
            for s in range(2):
                cc = (k % 2) * 1024 + s * 512
                out_ap = pT[p % 2][32 * r:32 * r + 8, s * 512:(s + 1) * 512]
                nc.tensor.matmul(out_ap, m_sb[:, 0:8],
                                 xt[0, k // 2][:, cc:cc + 512],
                                 start=True, stop=False)
                nc.tensor.matmul(out_ap, m_sb[:, 8:16],
                                 xt[1, k // 2][:, cc:cc + 512],
                                 start=False, stop=True)
            if k % 2 == 1 and k >= 3:
                emit_stats(k // 2 - 1)
        emit_stats(NCHUNK - 1)
        emit_evac(4, 3)
        emit_evac(5, 1)

        # ---------- pooled stats finalize (DVE); yb = [yavg | ymax] ------
        yb = stat.tile([128, 2, 2], f32, tag="yb", name="yb")
        for t in range(2):
            nc.vector.reduce_sum(ysum[:, t:t + 1], sums[:, t, :], axis=AX.X)
            nc.vector.tensor_scalar_mul(yb[:, t, 0:1], ysum[:, t:t + 1],
                                        1.0 / HW)
            nc.vector.tensor_copy(yb[:, t, 1:2], ymax[:, t:t + 1])

        # ---------- SE FC chain: both pooled branches in one pass --------
        att = stat.tile([128, 2], f32, tag="att", name="att")
        ph = psF.tile([16, 2], f32, tag="psF", name="ph")
        nc.tensor.matmul(ph[:], w1_sb[:, 0:16], yb[:, 0, :],
                         start=True, stop=False)
        nc.tensor.matmul(ph[:], w1_sb[:, 16:32], yb[:, 1, :],
                         start=False, stop=True)
        hb = stat.tile([16, 2], f32, tag="hb", name="hb")
        nc.scalar.activation(hb[:], ph[:], AF.Relu, bias=b1_sb[:])
        for t in range(2):
            pa = psF.tile([128, 2], f32, tag="psF", name=f"pa{t}")
            nc.tensor.matmul(pa[:], w2_sb[:, t * 128:(t + 1) * 128],
                             hb[:], start=True, stop=True)
            sg = stat.tile([128, 2], f32, tag=f"sg{t}", name=f"sg{t}")
            nc.scalar.activation(sg[:], pa[:], AF.Sigmoid,
                                 bias=b2_sb[:, t:t + 1])
            # att = sg_avg + sg_max > 0, so Relu (which takes an AP bias)
            # is a free add.
            nc.scalar.activation(att[:, t:t + 1], sg[:, 0:1], AF.Relu,
                                 bias=sg[:, 1:2])

        # ---------- diffusion, closed form over 3 steps ------------------
        nc.vector.tensor_copy(Tp[:, 0:1], Tp[:, 2:3])
        nc.vector.tensor_copy(Tp[:, W + 1:W + 2], Tp[:, W - 1:W])
        pd = psD.tile([128, W], f32, tag="psD", name="psD")
        cur = Tp
        for n in range(3):
            A = stat.tile([128, W], f32, tag=f"dA{n}", name=f"dA{n}")
            nc.vector.tensor_add(A[:], cur[:, 0:W], cur[:, 2:W + 2])
            U = stat.tile([128, W + 2], f32, tag=f"dU{n}", name=f"dU{n}")
            nc.vector.scalar_tensor_tensor(U[:, 1:W + 1], cur[:, 1:W + 1],
                                           4.0, A[:],
                                           op0=ALU.mult, op1=ALU.add)
            nc.tensor.matmul(pd[:], g_sb[n][:], U[:, 1:W + 1],
                             start=(n == 0), stop=(n == 2))
            if n < 2:
                nc.vector.tensor_copy(U[:, 0:1], U[:, 2:3])
                nc.vector.tensor_copy(U[:, W + 1:W + 2], U[:, W - 1:W])
            cur = U
        T3 = stat.tile([128, W], f32, tag="T3", name="T3")
        nc.vector.scalar_tensor_tensor(T3[:], Tp[:, 1:W + 1], c2p3, pd[:],
                                       op0=ALU.mult, op1=ALU.add)
        nc.scalar.activation(heat[:], T3[:], AF.Sigmoid)
        nc.sync.dma_start(out=hbuf[:], in_=heat[:])

        # ---------- Phase B: sigmoid(att*heat) * x ----------------------
        actx.close()  # free phase-A PSUM banks for psB
        with tc.tile_pool(name="psB", bufs=2, space="PSUM") as psB:
            for j in range(NCHUNK):
                pb = psB.tile([128, CH], f32, tag="psB", name="psB")
                for q in range(4):
                    nc.tensor.matmul(
                        pb[:, q * 512:(q + 1) * 512], on_sb[:],
                        hbuf[0:1, j * CH + q * 512:j * CH + (q + 1) * 512],
                        start=True, stop=True)
                for t in range(2):
                    sc = work.tile([128, CH], bf16, tag="sc", name="sc",
                                   bufs=3)
                    nc.scalar.activation(sc[:], pb[:], AF.Sigmoid,
                                         scale=att[:, t:t + 1])
                    ot = work.tile([128, CH], bf16, tag="ot", name="ot",
                                   bufs=3)
                    nc.vector.tensor_mul(ot[:], xv(t, j), sc[:])
                    # split the final writes so the drain tail is short
                    nsp = 4 if j == NCHUNK - 1 else (2 if j >= 5 else 1)
                    cw = CH // nsp
                    for u in range(nsp):
                        nc.sync.dma_start(
                            out=outd[t * 128:(t + 1) * 128,
                                     j * CH + u * cw:j * CH + (u + 1) * cw],
                            in_=ot[:, u * cw:(u + 1) * cw])

    nc.compile()
    return nc


_prog_cache = {}
_TRACE = False      # test harness sets True to collect an NTFF profile
_last_res = None    # BassKernelResults of the most recent run


def kernel(x, dct_w, w1, b1, w2, b2, alpha, lap):
    import ml_dtypes

    x = np.ascontiguousarray(np.asarray(x, dtype=np.float32))
    dct_w = np.asarray(dct_w, dtype=np.float32)
    w1 = np.asarray(w1, dtype=np.float32)
    b1 = np.asarray(b1, dtype=np.float32)
    w2 = np.asarray(w2, dtype=np.float32)
    b2 = np.asarray(b2, dtype=np.float32)
    alpha = float(np.asarray(alpha))
    lap = np.asarray(lap, dtype=np.float64)

    # decomposition requires the kernel's row structure (holds for HCFDA's
    # fixed Laplacian); verify.
    assert np.allclose(lap[0], lap[2]) and np.allclose(lap[:, 0], lap[:, 2])
    a, b = float(lap[0, 0]), float(lap[0, 1])
    assert abs(b / a - 4.0) < 1e-12  # Rh = S_l + S_r + (b/a) I, staged as 4
    c1 = alpha * float(lap[1, 0])
    c2 = 1.0 + alpha * (float(lap[1, 1]) - float(lap[1, 0]) * b / a)

    m = dct_w.astype(np.float64).mean(axis=0)           # [C]
    S = np.zeros((H, H), dtype=np.float64)
    for h in range(H):
        S[h, _reflect(h - 1, H)] += 1.0
        S[h, _reflect(h + 1, H)] += 1.0
    Rv = (alpha * a) * S + c1 * np.eye(H)
    # T3 = c2^3 T + sum_n C(3,n) c2^(3-n) Rv^n (Rh^n T)
    binom = {1: 3.0, 2: 3.0, 3: 1.0}
    gms = {}
    P = np.eye(H)
    for n in (1, 2, 3):
        P = P @ Rv
        Gn = binom[n] * (c2 ** (3 - n)) * P
        gms[n] = np.ascontiguousarray(Gn.T.astype(np.float32))

    m2 = m.reshape(2, 128).T                             # [128,2]
    mf = np.ascontiguousarray(
        np.repeat(m2, 8, axis=1).astype(ml_dtypes.bfloat16))  # [128,16]
    w1t = np.ascontiguousarray(
        w1.T.reshape(2, 128, 16).transpose(1, 0, 2).reshape(128, 32))
    w2t = np.ascontiguousarray(w2.T)                     # [16,256]
    b1c = np.ascontiguousarray(b1.reshape(16, 1))
    b2c = np.ascontiguousarray(b2.reshape(2, 128).T)     # [128,2]

    key = (c1, c2)
    if key not in _prog_cache:
        _prog_cache[key] = _build_program(c1, c2)
    nc = _prog_cache[key]

    consts = {"mf": mf, "gm0": gms[1], "gm1": gms[2], "gm2": gms[3],
              "w1t": w1t, "w2t": w2t,
              "b1c": b1c, "b2c": b2c,
              "onr": np.ones((1, 128), dtype=ml_dtypes.bfloat16)}
    xh = x.reshape(B, C, HW).astype(ml_dtypes.bfloat16)
    in_maps = [{"xb": xh[i], **consts} for i in range(N_CORES)]

    from concourse.bass_utils import run_bass_kernel_spmd
    res = run_bass_kernel_spmd(nc, in_maps, list(range(N_CORES)),
                               trace=_TRACE)
    global _last_res
    _last_res = res
    out = np.stack([res.results[i]["out"].astype(np.float32).reshape(C, H, W)
                    for i in range(N_CORES)])
    return out


# revision 30
# speedup vs baseline: 1.0732x; 1.0732x over previous
"""Trainium2 Bass kernel for the HCFDA dense-CNN module (bf16 I/O).

Math used (exact reassociations of the reference):
  1. The 256x256 1x1 DCT conv is only consumed through a channel-mean, so
     temp[b,h,w] = sum_c m[c] * x[b,c,h,w]  with  m = dct_w.mean(axis=0).
  2. The 3-step diffusion is a polynomial in commuting 1-D reflect-shift
     operators:  T' = c2*T + Rv(Rh T)  with  Rh = S_l + S_r + 4 I (cols,
     free axis) and Rv = alpha*a*(S_u + S_d) + c1*I (rows, via PE matmul).
     Hence T3 = c2^3*T + sum_n C(3,n) c2^(3-n) Rv^n (Rh^n T), n=1..3,
     with Rv^n staged as three precomputed 128x128 lhsT matrices.
  3. SE branch: pooled stats -> two tiny FCs -> sigmoid, per reference.
  out = x * sigmoid(att[c] * sigmoid(T3)[h,w])

Performance structure (per core; tolerance 2e-2 allows bf16 I/O, which
halves HBM traffic vs f32 — all accumulation stays f32):
  - Pooled stats split across engines: ACT Copy+accum_out sums for early
    chunks plus tree roots; DVE pairwise add/max trees (TensorTensor gets
    the 2x packed mode; TensorReduce does not, so trees beat flat
    reduces) with only two final reduces.
  - GEMV temp runs m-stationary (replicated 8-wide) in bf16 on the PE;
    chunks pack 3-per-PSUM-tile at partition bases {0,32,64}; ACT
    evacuates each pack with one Copy, a reshape DMA lands rows in Tp.
  - The 3-step diffusion is applied in closed form (one DVE shift-chain
    per power of Rh, three accumulated PE matmuls, one STT).
  - Phase B: PE ones-matmul broadcasts heat rows to all partitions; ACT
    fuses sigmoid(att*heat) via per-partition scale (the pacing engine);
    DVE multiplies with resident bf16 x; DMA-out rides underneath.

Sharding: pure data parallel, one batch element per NeuronCore (B=8).
"""

import numpy as np
from contextlib import ExitStack

B, C, H, W = 8, 256, 128, 128
HW = H * W           # 16384
NCHUNK = 8           # x chunks over HW
CH = HW // NCHUNK    # 2048
N_CORES = 8


def _reflect(i, n):
    if i < 0:
        return -i
    if i >= n:
        return 2 * (n - 1) - i
    return i


def _build_program(c1, c2):
    from concourse import bass, mybir, tile
    from concourse import bacc

    f32 = mybir.dt.float32
    bf16 = mybir.dt.bfloat16
    AF = mybir.ActivationFunctionType
    ALU = mybir.AluOpType
    AX = mybir.AxisListType
    c2p3 = c2 * c2 * c2

    nc = bacc.Bacc("TRN2", target_bir_lowering=False, debug=False,
                   num_devices=N_CORES)

    xb = nc.dram_tensor("xb", [C, HW], bf16, kind="ExternalInput").ap()
    mf = nc.dram_tensor("mf", [128, 16], bf16, kind="ExternalInput").ap()
    gmd = [nc.dram_tensor(f"gm{n}", [128, 128], f32,
                          kind="ExternalInput").ap() for n in range(3)]
    w1d = nc.dram_tensor("w1t", [128, 32], f32, kind="ExternalInput").ap()
    w2d = nc.dram_tensor("w2t", [16, 256], f32, kind="ExternalInput").ap()
    b1d = nc.dram_tensor("b1c", [16, 1], f32, kind="ExternalInput").ap()
    b2d = nc.dram_tensor("b2c", [128, 2], f32, kind="ExternalInput").ap()
    ond = nc.dram_tensor("onr", [1, 128], bf16, kind="ExternalInput").ap()
    outd = nc.dram_tensor("out", [C, HW], bf16, kind="ExternalOutput").ap()

    with tile.TileContext(nc) as tc, ExitStack() as ctx:
        const = ctx.enter_context(tc.tile_pool(name="const", bufs=1))
        xpool = ctx.enter_context(tc.tile_pool(name="xp", bufs=1))
        work = ctx.enter_context(tc.tile_pool(name="work", bufs=2))
        stat = ctx.enter_context(tc.tile_pool(name="stat", bufs=1))
        actx = ctx.enter_context(ExitStack())
        psT = actx.enter_context(tc.tile_pool(name="psT", bufs=1, space="PSUM"))
        psD = actx.enter_context(tc.tile_pool(name="psD", bufs=1, space="PSUM"))
        psF = actx.enter_context(tc.tile_pool(name="psF", bufs=2, space="PSUM"))

        m_sb = const.tile([128, 16], bf16, tag="m", name="m")
        nc.sync.dma_start(out=m_sb[:], in_=mf)
        g_sb = []
        for n in range(3):
            g = const.tile([128, 128], f32, tag=f"g{n}", name=f"g{n}")
            nc.sync.dma_start(out=g[:], in_=gmd[n])
            g_sb.append(g)
        w1_sb = const.tile([128, 32], f32, tag="w1", name="w1")
        nc.sync.dma_start(out=w1_sb[:], in_=w1d)
        w2_sb = const.tile([16, 256], f32, tag="w2", name="w2")
        nc.sync.dma_start(out=w2_sb[:], in_=w2d)
        b1_sb = const.tile([16, 1], f32, tag="b1", name="b1")
        nc.sync.dma_start(out=b1_sb[:], in_=b1d)
        b2_sb = const.tile([128, 2], f32, tag="b2", name="b2")
        nc.sync.dma_start(out=b2_sb[:], in_=b2d)
        on_sb = const.tile([1, 128], bf16, tag="onr", name="onr")
        nc.sync.dma_start(out=on_sb[:], in_=ond)
        warm = const.tile([1, 2], f32, tag="warm", name="warm")
        nc.scalar.activation(warm[:], b2_sb[0:1, 0:2], AF.Sigmoid)

        sums = stat.tile([128, 2, 6], f32, tag="sums", name="sums")
        ms = stat.tile([128, 2, 2], f32, tag="ms", name="ms")
        junk = stat.tile([128, CH], bf16, tag="junk", name="junk")
        heat = stat.tile([128, W], bf16, tag="heat", name="heat")
        hbuf = stat.tile([1, HW], bf16, tag="hbuf", name="hbuf")
        Tp = stat.tile([128, W + 2], f32, tag="Tp", name="Tp")
        ymax = stat.tile([128, 2], f32, tag="ymax", name="ymax")
        ysum = stat.tile([128, 2], f32, tag="ysum", name="ysum")
        # pairwise-tree temporaries
        gmx = {(t, i): stat.tile([128, CH], bf16, tag=f"g{t}{i}",
                                 name=f"g{t}{i}")
               for t in range(2) for i in range(3)}
        ta = [stat.tile([128, CH], bf16, tag=f"ta{t}", name=f"ta{t}")
              for t in range(2)]

        # ---------- x loads (SP-issued, one DMA per channel-half chunk;
        # the first chunk is split into halves so compute starts sooner) --
        xt = {}
        for j in range(NCHUNK):
            for t in range(2):
                xt[t, j] = xpool.tile([128, CH], bf16, tag=f"x{t}_{j}",
                                      name=f"x{t}_{j}")
                nsp = 2 if j == 0 else 1
                cw = CH // nsp
                for q in range(nsp):
                    nc.sync.dma_start(
                        out=xt[t, j][:, q * cw:(q + 1) * cw],
                        in_=xb[t * 128:(t + 1) * 128,
                               j * CH + q * cw:j * CH + (q + 1) * cw])

        def xv(t, j):
            return xt[t, j][:]

        # ---------- phase A: GEMV (bf16) + pooled stats -------------------
        pT = [psT.tile([128, 1024], f32, tag=f"psT{i}", name=f"psT{i}")
              for i in range(2)]

        def emit_evac(p, nk):
            S3 = work.tile([128, 1024], f32, tag="s3", name="s3", bufs=2)
            nc.scalar.activation(S3[0:32 * (nk - 1) + 8, :],
                                 pT[p % 2][0:32 * (nk - 1) + 8, :],
                                 AF.Copy)
            sv = S3[:].rearrange("(a b) f -> a b f", a=4, b=32)
            nc.sync.dma_start(out=Tp[24 * p:24 * p + 8 * nk, 1:W + 1],
                              in_=sv[0:nk, 0:1, :])

        def emit_stats(j):
            # sums: flat ACT accums for j<=4, a short DVE tree for 5..7.
            # maxs: running DVE tree arranged so chunks 0..6 are fully
            # reduced before chunk 7 lands; only its own reduce trails.
            for t in range(2):
                if j <= 4:
                    nc.scalar.activation(junk[:], xv(t, j), AF.Copy,
                                         accum_out=sums[:, t, j:j + 1])
                if j in (1, 3, 5):
                    i = (j - 1) // 2
                    nc.vector.tensor_max(gmx[t, i][:], xv(t, j - 1),
                                         xv(t, j))
                if j == 3:
                    nc.vector.tensor_max(gmx[t, 0][:], gmx[t, 0][:],
                                         gmx[t, 1][:])
                if j == 6:
                    nc.vector.tensor_max(gmx[t, 2][:], gmx[t, 2][:],
                                         xv(t, 6))
                    nc.vector.tensor_max(gmx[t, 0][:], gmx[t, 0][:],
                                         gmx[t, 2][:])
                    nc.vector.reduce_max(ms[:, t, 0:1], gmx[t, 0][:],
                                         axis=AX.X)
                    nc.vector.tensor_add(ta[t][:], xv(t, 5), xv(t, 6))
                if j == 7:
                    nc.vector.reduce_max(ms[:, t, 1:2], xv(t, 7), axis=AX.X)
                    nc.vector.tensor_max(ymax[:, t:t + 1], ms[:, t, 0:1],
                                         ms[:, t, 1:2])
                    nc.vector.tensor_add(ta[t][:], ta[t][:], xv(t, 7))
                    nc.scalar.activation(junk[:], ta[t][:], AF.Copy,
                                         accum_out=sums[:, t, 5:6])

        for k in range(16):   # temp chunks of 1024 cols
            p, r = divmod(k, 3)
            if r == 0 and p >= 2:
                emit_evac(p - 2, 3)
            for s in range(2):
                cc = (k % 2) * 1024 + s * 512
                out_ap = pT[p % 2][32 * r:32 * r + 8, s * 512:(s + 1) * 512]
                nc.tensor.matmul(out_ap, m_sb[:, 0:8],
                                 xt[0, k // 2][:, cc:cc + 512],
                                 start=True, stop=False)
                nc.tensor.matmul(out_ap, m_sb[:, 8:16],
                                 xt[1, k // 2][:, cc:cc + 512],
                                 start=False, stop=True)
            if k % 2 == 1 and k >= 3:
                emit_stats(k // 2 - 1)
        emit_stats(NCHUNK - 1)
        emit_evac(4, 3)
        emit_evac(5, 1)

        # ---------- pooled stats finalize (DVE); yb = [yavg | ymax] ------
        yb = stat.tile([128, 2, 2], f32, tag="yb", name="yb")
        for t in range(2):
            nc.vector.reduce_sum(ysum[:, t:t + 1], sums[:, t, :], axis=AX.X)
            nc.vector.tensor_scalar_mul(yb[:, t, 0:1], ysum[:, t:t + 1],
                                        1.0 / HW)
            nc.vector.tensor_copy(yb[:, t, 1:2], ymax[:, t:t + 1])

        # ---------- SE FC chain: both pooled branches in one pass --------
        att = stat.tile([128, 2], f32, tag="att", name="att")
        ph = psF.tile([16, 2], f32, tag="psF", name="ph")
        nc.tensor.matmul(ph[:], w1_sb[:, 0:16], yb[:, 0, :],
                         start=True, stop=False)
        nc.tensor.matmul(ph[:], w1_sb[:, 16:32], yb[:, 1, :],
                         start=False, stop=True)
        hb = stat.tile([16, 2], f32, tag="hb", name="hb")
        nc.scalar.activation(hb[:], ph[:], AF.Relu, bias=b1_sb[:])
        for t in range(2):
            pa = psF.tile([128, 2], f32, tag="psF", name=f"pa{t}")
            nc.tensor.matmul(pa[:], w2_sb[:, t * 128:(t + 1) * 128],
                             hb[:], start=True, stop=True)
            sg = stat.tile([128, 2], f32, tag=f"sg{t}", name=f"sg{t}")
            nc.scalar.activation(sg[:], pa[:], AF.Sigmoid,
                                 bias=b2_sb[:, t:t + 1])
            # att = sg_avg + sg_max > 0, so Relu (which takes an AP bias)
            # is a free add.
            nc.scalar.activation(att[:, t:t + 1], sg[:, 0:1], AF.Relu,
                                 bias=sg[:, 1:2])

        # ---------- diffusion, closed form over 3 steps ------------------
        nc.vector.tensor_copy(Tp[:, 0:1], Tp[:, 2:3])
        nc.vector.tensor_copy(Tp[:, W + 1:W + 2], Tp[:, W - 1:W])
        pd = psD.tile([128, W], f32, tag="psD", name="psD")
        cur = Tp
        for n in range(3):
            A = stat.tile([128, W], f32, tag=f"dA{n}", name=f"dA{n}")
            nc.vector.tensor_add(A[:], cur[:, 0:W], cur[:, 2:W + 2])
            U = stat.tile([128, W + 2], f32, tag=f"dU{n}", name=f"dU{n}")
            nc.vector.scalar_tensor_tensor(U[:, 1:W + 1], cur[:, 1:W + 1],
                                           4.0, A[:],
                                           op0=ALU.mult, op1=ALU.add)
            nc.tensor.matmul(pd[:], g_sb[n][:], U[:, 1:W + 1],
                             start=(n == 0), stop=(n == 2))
            if n < 2:
                nc.vector.tensor_copy(U[:, 0:1], U[:, 2:3])
                nc.vector.tensor_copy(U[:, W + 1:W + 2], U[:, W - 1:W])
            cur = U
        T3 = stat.tile([128, W], f32, tag="T3", name="T3")
        nc.vector.scalar_tensor_tensor(T3[:], Tp[:, 1:W + 1], c2p3, pd[:],
                                       op0=ALU.mult, op1=ALU.add)
        nc.scalar.activation(heat[:], T3[:], AF.Sigmoid)
        nc.sync.dma_start(out=hbuf[:], in_=heat[:])

        # ---------- Phase B: sigmoid(att*heat) * x ----------------------
        actx.close()  # free phase-A PSUM banks for psB
        with tc.tile_pool(name="psB", bufs=2, space="PSUM") as psB:
            for j in range(NCHUNK):
                pb = psB.tile([128, CH], f32, tag="psB", name="psB")
                for q in range(4):
                    nc.tensor.matmul(
                        pb[:, q * 512:(q + 1) * 512], on_sb[:],
                        hbuf[0:1, j * CH + q * 512:j * CH + (q + 1) * 512],
                        start=True, stop=True)
                for t in range(2):
                    sc = work.tile([128, CH], bf16, tag="sc", name="sc",
                                   bufs=3)
                    nc.scalar.activation(sc[:], pb[:], AF.Sigmoid,
                                         scale=att[:, t:t + 1])
                    ot = work.tile([128, CH], bf16, tag="ot", name="ot",
                                   bufs=3)
                    nc.vector.tensor_mul(ot[:], xv(t, j), sc[:])
                    # split the final writes so the drain tail is short
                    nsp = 4 if j == NCHUNK - 1 else (2 if j >= 5 else 1)
                    cw = CH // nsp
                    for u in range(nsp):
                        nc.sync.dma_start(
                            out=outd[t * 128:(t + 1) * 128,
                                     j * CH + u * cw:j * CH + (u + 1) * cw],
                            in_=ot[:, u * cw:(u + 1) * cw])

    nc.compile()
    return nc


_prog_cache = {}
_TRACE = False      # test harness sets True to collect an NTFF profile
_last_res = None    # BassKernelResults of the most recent run


def kernel(x, dct_w, w1, b1, w2, b2, alpha, lap):
    import ml_dtypes

    x = np.ascontiguousarray(np.asarray(x, dtype=np.float32))
    dct_w = np.asarray(dct_w, dtype=np.float32)
    w1 = np.asarray(w1, dtype=np.float32)
    b1 = np.asarray(b1, dtype=np.float32)
    w2 = np.asarray(w2, dtype=np.float32)
    b2 = np.asarray(b2, dtype=np.float32)
    alpha = float(np.asarray(alpha))
    lap = np.asarray(lap, dtype=np.float64)

    # decomposition requires the kernel's row structure (holds for HCFDA's
    # fixed Laplacian); verify.
    assert np.allclose(lap[0], lap[2]) and np.allclose(lap[:, 0], lap[:, 2])
    a, b = float(lap[0, 0]), float(lap[0, 1])
    assert abs(b / a - 4.0) < 1e-12  # Rh = S_l + S_r + (b/a) I, staged as 4
    c1 = alpha * float(lap[1, 0])
    c2 = 1.0 + alpha * (float(lap[1, 1]) - float(lap[1, 0]) * b / a)

    m = dct_w.astype(np.float64).mean(axis=0)           # [C]
    S = np.zeros((H, H), dtype=np.float64)
    for h in range(H):
        S[h, _reflect(h - 1, H)] += 1.0
        S[h, _reflect(h + 1, H)] += 1.0
    Rv = (alpha * a) * S + c1 * np.eye(H)
    # T3 = c2^3 T + sum_n C(3,n) c2^(3-n) Rv^n (Rh^n T)
    binom = {1: 3.0, 2: 3.0, 3: 1.0}
    gms = {}
    P = np.eye(H)
    for n in (1, 2, 3):
        P = P @ Rv
        Gn = binom[n] * (c2 ** (3 - n)) * P
        gms[n] = np.ascontiguousarray(Gn.T.astype(np.float32))

    m2 = m.reshape(2, 128).T                             # [128,2]
    mf = np.ascontiguousarray(
        np.repeat(m2, 8, axis=1).astype(ml_dtypes.bfloat16))  # [128,16]
    w1t = np.ascontiguousarray(
        w1.T.reshape(2, 128, 16).transpose(1, 0, 2).reshape(128, 32))
    w2t = np.ascontiguousarray(w2.T)                     # [16,256]
    b1c = np.ascontiguousarray(b1.reshape(16, 1))
    b2c = np.ascontiguousarray(b2.reshape(2, 128).T)     # [128,2]

    key = (c1, c2)
    if key not in _prog_cache:
        _prog_cache[key] = _build_program(c1, c2)
    nc = _prog_cache[key]

    consts = {"mf": mf, "gm0": gms[1], "gm1": gms[2], "gm2": gms[3],
              "w1t": w1t, "w2t": w2t,
              "b1c": b1c, "b2c": b2c,
              "onr": np.ones((1, 128), dtype=ml_dtypes.bfloat16)}
    xh = x.reshape(B, C, HW).astype(ml_dtypes.bfloat16)
    in_maps = [{"xb": xh[i], **consts} for i in range(N_CORES)]

    from concourse.bass_utils import run_bass_kernel_spmd
    res = run_bass_kernel_spmd(nc, in_maps, list(range(N_CORES)),
                               trace=_TRACE)
    global _last_res
    _last_res = res
    out = np.stack([res.results[i]["out"].astype(np.float32).reshape(C, H, W)
                    for i in range(N_CORES)])
    return out


# revision 31
# speedup vs baseline: 1.0983x; 1.0233x over previous
"""Trainium2 Bass kernel for the HCFDA dense-CNN module (bf16 I/O).

Math used (exact reassociations of the reference):
  1. The 256x256 1x1 DCT conv is only consumed through a channel-mean, so
     temp[b,h,w] = sum_c m[c] * x[b,c,h,w]  with  m = dct_w.mean(axis=0).
  2. The 3-step diffusion is a polynomial in commuting 1-D reflect-shift
     operators:  T' = c2*T + Rv(Rh T)  with  Rh = S_l + S_r + 4 I (cols,
     free axis) and Rv = alpha*a*(S_u + S_d) + c1*I (rows, via PE matmul).
     Hence T3 = c2^3*T + sum_n C(3,n) c2^(3-n) Rv^n (Rh^n T), n=1..3,
     with Rv^n staged as three precomputed 128x128 lhsT matrices.
  3. SE branch: pooled stats -> two tiny FCs -> sigmoid, per reference.
  out = x * sigmoid(att[c] * sigmoid(T3)[h,w])

Performance structure (per core; tolerance 2e-2 allows bf16 I/O, which
halves HBM traffic vs f32 — all accumulation stays f32):
  - Pooled stats split across engines: ACT Copy+accum_out sums for early
    chunks plus tree roots; DVE pairwise add/max trees (TensorTensor gets
    the 2x packed mode; TensorReduce does not, so trees beat flat
    reduces) with only two final reduces.
  - GEMV temp runs m-stationary (replicated 8-wide) in bf16 on the PE;
    chunks pack 3-per-PSUM-tile at partition bases {0,32,64}; ACT
    evacuates each pack with one Copy, a reshape DMA lands rows in Tp.
  - The 3-step diffusion is applied in closed form (one DVE shift-chain
    per power of Rh, three accumulated PE matmuls, one STT).
  - Phase B: PE ones-matmul broadcasts heat rows to all partitions; ACT
    fuses sigmoid(att*heat) via per-partition scale (the pacing engine);
    DVE multiplies with resident bf16 x; DMA-out rides underneath.

Sharding: pure data parallel, one batch element per NeuronCore (B=8).
"""

import numpy as np
from contextlib import ExitStack

B, C, H, W = 8, 256, 128, 128
HW = H * W           # 16384
NCHUNK = 8           # x chunks over HW
CH = HW // NCHUNK    # 2048
N_CORES = 8


def _reflect(i, n):
    if i < 0:
        return -i
    if i >= n:
        return 2 * (n - 1) - i
    return i


def _build_program(c1, c2):
    from concourse import bass, mybir, tile
    from concourse import bacc

    f32 = mybir.dt.float32
    bf16 = mybir.dt.bfloat16
    AF = mybir.ActivationFunctionType
    ALU = mybir.AluOpType
    AX = mybir.AxisListType
    c2p3 = c2 * c2 * c2

    nc = bacc.Bacc("TRN2", target_bir_lowering=False, debug=False,
                   num_devices=N_CORES)

    xb = nc.dram_tensor("xb", [C, HW], bf16, kind="ExternalInput").ap()
    mf = nc.dram_tensor("mf", [128, 16], bf16, kind="ExternalInput").ap()
    gmd = [nc.dram_tensor(f"gm{n}", [128, 128], f32,
                          kind="ExternalInput").ap() for n in range(3)]
    w1d = nc.dram_tensor("w1t", [128, 32], f32, kind="ExternalInput").ap()
    w2d = nc.dram_tensor("w2t", [16, 256], f32, kind="ExternalInput").ap()
    b1d = nc.dram_tensor("b1c", [16, 1], f32, kind="ExternalInput").ap()
    b2d = nc.dram_tensor("b2c", [128, 2], f32, kind="ExternalInput").ap()
    ond = nc.dram_tensor("onr", [1, 128], bf16, kind="ExternalInput").ap()
    outd = nc.dram_tensor("out", [C, HW], bf16, kind="ExternalOutput").ap()

    with tile.TileContext(nc) as tc, ExitStack() as ctx:
        const = ctx.enter_context(tc.tile_pool(name="const", bufs=1))
        xpool = ctx.enter_context(tc.tile_pool(name="xp", bufs=1))
        work = ctx.enter_context(tc.tile_pool(name="work", bufs=2))
        stat = ctx.enter_context(tc.tile_pool(name="stat", bufs=1))
        actx = ctx.enter_context(ExitStack())
        psT = actx.enter_context(tc.tile_pool(name="psT", bufs=1, space="PSUM"))
        psD = actx.enter_context(tc.tile_pool(name="psD", bufs=1, space="PSUM"))
        psF = actx.enter_context(tc.tile_pool(name="psF", bufs=2, space="PSUM"))

        # m first (the GEMV needs it within ~10us); bulk consts are
        # issued after the x-chunk DMAs so they don't delay the stream.
        m_sb = const.tile([128, 16], bf16, tag="m", name="m")
        nc.sync.dma_start(out=m_sb[:], in_=mf)
        warm = const.tile([1, 2], f32, tag="warm", name="warm")
        nc.scalar.activation(warm[:], m_sb[0:1, 0:2], AF.Sigmoid)

        sums = stat.tile([128, 2, 6], f32, tag="sums", name="sums")
        ms = stat.tile([128, 2, 2], f32, tag="ms", name="ms")
        junk = stat.tile([128, CH], bf16, tag="junk", name="junk")
        heat = stat.tile([128, W], bf16, tag="heat", name="heat")
        hbuf = stat.tile([1, HW], bf16, tag="hbuf", name="hbuf")
        Tp = stat.tile([128, W + 2], f32, tag="Tp", name="Tp")
        ymax = stat.tile([128, 2], f32, tag="ymax", name="ymax")
        ysum = stat.tile([128, 2], f32, tag="ysum", name="ysum")
        # pairwise-tree temporaries
        gmx = {(t, i): stat.tile([128, CH], bf16, tag=f"g{t}{i}",
                                 name=f"g{t}{i}")
               for t in range(2) for i in range(3)}
        ta = [stat.tile([128, CH], bf16, tag=f"ta{t}", name=f"ta{t}")
              for t in range(2)]

        # ---------- x loads (SP-issued, one DMA per channel-half chunk;
        # the first chunk is split into halves so compute starts sooner) --
        xt = {}
        for j in range(NCHUNK):
            for t in range(2):
                xt[t, j] = xpool.tile([128, CH], bf16, tag=f"x{t}_{j}",
                                      name=f"x{t}_{j}")
                nsp = 2 if j < 3 else 1
                cw = CH // nsp
                for q in range(nsp):
                    nc.sync.dma_start(
                        out=xt[t, j][:, q * cw:(q + 1) * cw],
                        in_=xb[t * 128:(t + 1) * 128,
                               j * CH + q * cw:j * CH + (q + 1) * cw])

        g_sb = []
        for n in range(3):
            g = const.tile([128, 128], f32, tag=f"g{n}", name=f"g{n}")
            nc.sync.dma_start(out=g[:], in_=gmd[n])
            g_sb.append(g)
        w1_sb = const.tile([128, 32], f32, tag="w1", name="w1")
        nc.sync.dma_start(out=w1_sb[:], in_=w1d)
        w2_sb = const.tile([16, 256], f32, tag="w2", name="w2")
        nc.sync.dma_start(out=w2_sb[:], in_=w2d)
        b1_sb = const.tile([16, 1], f32, tag="b1", name="b1")
        nc.sync.dma_start(out=b1_sb[:], in_=b1d)
        b2_sb = const.tile([128, 2], f32, tag="b2", name="b2")
        nc.sync.dma_start(out=b2_sb[:], in_=b2d)
        on_sb = const.tile([1, 128], bf16, tag="onr", name="onr")
        nc.sync.dma_start(out=on_sb[:], in_=ond)

        def xv(t, j):
            return xt[t, j][:]

        # ---------- phase A: GEMV (bf16) + pooled stats -------------------
        pT = [psT.tile([128, 1024], f32, tag=f"psT{i}", name=f"psT{i}")
              for i in range(2)]

        def emit_evac(p, nk):
            S3 = work.tile([128, 1024], f32, tag="s3", name="s3", bufs=2)
            nc.scalar.activation(S3[0:32 * (nk - 1) + 8, :],
                                 pT[p % 2][0:32 * (nk - 1) + 8, :],
                                 AF.Copy)
            sv = S3[:].rearrange("(a b) f -> a b f", a=4, b=32)
            nc.sync.dma_start(out=Tp[24 * p:24 * p + 8 * nk, 1:W + 1],
                              in_=sv[0:nk, 0:1, :])

        def emit_stats(j):
            # sums: flat ACT accums for j<=4, a short DVE tree for 5..7.
            # maxs: running DVE tree arranged so chunks 0..6 are fully
            # reduced before chunk 7 lands; only its own reduce trails.
            for t in range(2):
                if j <= 4:
                    nc.scalar.activation(junk[:], xv(t, j), AF.Copy,
                                         accum_out=sums[:, t, j:j + 1])
                if j in (1, 3, 5):
                    i = (j - 1) // 2
                    nc.vector.tensor_max(gmx[t, i][:], xv(t, j - 1),
                                         xv(t, j))
                if j == 3:
                    nc.vector.tensor_max(gmx[t, 0][:], gmx[t, 0][:],
                                         gmx[t, 1][:])
                if j == 6:
                    nc.vector.tensor_max(gmx[t, 2][:], gmx[t, 2][:],
                                         xv(t, 6))
                    nc.vector.tensor_max(gmx[t, 0][:], gmx[t, 0][:],
                                         gmx[t, 2][:])
                    nc.vector.reduce_max(ms[:, t, 0:1], gmx[t, 0][:],
                                         axis=AX.X)
                    nc.vector.tensor_add(ta[t][:], xv(t, 5), xv(t, 6))
                if j == 7:
                    nc.vector.reduce_max(ms[:, t, 1:2], xv(t, 7), axis=AX.X)
                    nc.vector.tensor_max(ymax[:, t:t + 1], ms[:, t, 0:1],
                                         ms[:, t, 1:2])
                    nc.vector.tensor_add(ta[t][:], ta[t][:], xv(t, 7))
                    nc.scalar.activation(junk[:], ta[t][:], AF.Copy,
                                         accum_out=sums[:, t, 5:6])

        for k in range(16):   # temp chunks of 1024 cols
            p, r = divmod(k, 3)
            if r == 0 and p >= 2:
                emit_evac(p - 2, 3)
            for s in range(2):
                cc = (k % 2) * 1024 + s * 512
                out_ap = pT[p % 2][32 * r:32 * r + 8, s * 512:(s + 1) * 512]
                nc.tensor.matmul(out_ap, m_sb[:, 0:8],
                                 xt[0, k // 2][:, cc:cc + 512],
                                 start=True, stop=False)
                nc.tensor.matmul(out_ap, m_sb[:, 8:16],
                                 xt[1, k // 2][:, cc:cc + 512],
                                 start=False, stop=True)
            if k % 2 == 1 and k >= 3:
                emit_stats(k // 2 - 1)
        emit_stats(NCHUNK - 1)
        emit_evac(4, 3)
        emit_evac(5, 1)

        # ---------- pooled stats finalize (DVE); yb = [yavg | ymax] ------
        yb = stat.tile([128, 2, 2], f32, tag="yb", name="yb")
        for t in range(2):
            nc.vector.reduce_sum(ysum[:, t:t + 1], sums[:, t, :], axis=AX.X)
            nc.vector.tensor_scalar_mul(yb[:, t, 0:1], ysum[:, t:t + 1],
                                        1.0 / HW)
            nc.vector.tensor_copy(yb[:, t, 1:2], ymax[:, t:t + 1])

        # ---------- SE FC chain: both pooled branches in one pass --------
        att = stat.tile([128, 2], f32, tag="att", name="att")
        ph = psF.tile([16, 2], f32, tag="psF", name="ph")
        nc.tensor.matmul(ph[:], w1_sb[:, 0:16], yb[:, 0, :],
                         start=True, stop=False)
        nc.tensor.matmul(ph[:], w1_sb[:, 16:32], yb[:, 1, :],
                         start=False, stop=True)
        hb = stat.tile([16, 2], f32, tag="hb", name="hb")
        nc.scalar.activation(hb[:], ph[:], AF.Relu, bias=b1_sb[:])
        for t in range(2):
            pa = psF.tile([128, 2], f32, tag="psF", name=f"pa{t}")
            nc.tensor.matmul(pa[:], w2_sb[:, t * 128:(t + 1) * 128],
                             hb[:], start=True, stop=True)
            sg = stat.tile([128, 2], f32, tag=f"sg{t}", name=f"sg{t}")
            nc.scalar.activation(sg[:], pa[:], AF.Sigmoid,
                                 bias=b2_sb[:, t:t + 1])
            # att = sg_avg + sg_max > 0, so Relu (which takes an AP bias)
            # is a free add.
            nc.scalar.activation(att[:, t:t + 1], sg[:, 0:1], AF.Relu,
                                 bias=sg[:, 1:2])

        # ---------- diffusion, closed form over 3 steps ------------------
        nc.vector.tensor_copy(Tp[:, 0:1], Tp[:, 2:3])
        nc.vector.tensor_copy(Tp[:, W + 1:W + 2], Tp[:, W - 1:W])
        pd = psD.tile([128, W], f32, tag="psD", name="psD")
        cur = Tp
        for n in range(3):
            A = stat.tile([128, W], f32, tag=f"dA{n}", name=f"dA{n}")
            nc.vector.tensor_add(A[:], cur[:, 0:W], cur[:, 2:W + 2])
            U = stat.tile([128, W + 2], f32, tag=f"dU{n}", name=f"dU{n}")
            nc.vector.scalar_tensor_tensor(U[:, 1:W + 1], cur[:, 1:W + 1],
                                           4.0, A[:],
                                           op0=ALU.mult, op1=ALU.add)
            nc.tensor.matmul(pd[:], g_sb[n][:], U[:, 1:W + 1],
                             start=(n == 0), stop=(n == 2))
            if n < 2:
                nc.vector.tensor_copy(U[:, 0:1], U[:, 2:3])
                nc.vector.tensor_copy(U[:, W + 1:W + 2], U[:, W - 1:W])
            cur = U
        T3 = stat.tile([128, W], f32, tag="T3", name="T3")
        nc.vector.scalar_tensor_tensor(T3[:], Tp[:, 1:W + 1], c2p3, pd[:],
                                       op0=ALU.mult, op1=ALU.add)
        nc.scalar.activation(heat[:], T3[:], AF.Sigmoid)
        nc.sync.dma_start(out=hbuf[:], in_=heat[:])

        # ---------- Phase B: sigmoid(att*heat) * x ----------------------
        actx.close()  # free phase-A PSUM banks for psB
        with tc.tile_pool(name="psB", bufs=2, space="PSUM") as psB:
            for j in range(NCHUNK):
                pb = psB.tile([128, CH], f32, tag="psB", name="psB")
                for q in range(4):
                    nc.tensor.matmul(
                        pb[:, q * 512:(q + 1) * 512], on_sb[:],
                        hbuf[0:1, j * CH + q * 512:j * CH + (q + 1) * 512],
                        start=True, stop=True)
                for t in range(2):
                    sc = work.tile([128, CH], bf16, tag="sc", name="sc",
                                   bufs=3)
                    nc.scalar.activation(sc[:], pb[:], AF.Sigmoid,
                                         scale=att[:, t:t + 1])
                    ot = work.tile([128, CH], bf16, tag="ot", name="ot",
                                   bufs=3)
                    nc.vector.tensor_mul(ot[:], xv(t, j), sc[:])
                    # split the final writes so the drain tail is short
                    nsp = 4 if j == NCHUNK - 1 else (2 if j >= 5 else 1)
                    cw = CH // nsp
                    for u in range(nsp):
                        nc.sync.dma_start(
                            out=outd[t * 128:(t + 1) * 128,
                                     j * CH + u * cw:j * CH + (u + 1) * cw],
                            in_=ot[:, u * cw:(u + 1) * cw])

    nc.compile()
    return nc


_prog_cache = {}
_TRACE = False      # test harness sets True to collect an NTFF profile
_last_res = None    # BassKernelResults of the most recent run


def kernel(x, dct_w, w1, b1, w2, b2, alpha, lap):
    import ml_dtypes

    x = np.ascontiguousarray(np.asarray(x, dtype=np.float32))
    dct_w = np.asarray(dct_w, dtype=np.float32)
    w1 = np.asarray(w1, dtype=np.float32)
    b1 = np.asarray(b1, dtype=np.float32)
    w2 = np.asarray(w2, dtype=np.float32)
    b2 = np.asarray(b2, dtype=np.float32)
    alpha = float(np.asarray(alpha))
    lap = np.asarray(lap, dtype=np.float64)

    # decomposition requires the kernel's row structure (holds for HCFDA's
    # fixed Laplacian); verify.
    assert np.allclose(lap[0], lap[2]) and np.allclose(lap[:, 0], lap[:, 2])
    a, b = float(lap[0, 0]), float(lap[0, 1])
    assert abs(b / a - 4.0) < 1e-12  # Rh = S_l + S_r + (b/a) I, staged as 4
    c1 = alpha * float(lap[1, 0])
    c2 = 1.0 + alpha * (float(lap[1, 1]) - float(lap[1, 0]) * b / a)

    m = dct_w.astype(np.float64).mean(axis=0)           # [C]
    S = np.zeros((H, H), dtype=np.float64)
    for h in range(H):
        S[h, _reflect(h - 1, H)] += 1.0
        S[h, _reflect(h + 1, H)] += 1.0
    Rv = (alpha * a) * S + c1 * np.eye(H)
    # T3 = c2^3 T + sum_n C(3,n) c2^(3-n) Rv^n (Rh^n T)
    binom = {1: 3.0, 2: 3.0, 3: 1.0}
    gms = {}
    P = np.eye(H)
    for n in (1, 2, 3):
        P = P @ Rv
        Gn = binom[n] * (c2 ** (3 - n)) * P
        gms[n] = np.ascontiguousarray(Gn.T.astype(np.float32))

    m2 = m.reshape(2, 128).T                             # [128,2]
    mf = np.ascontiguousarray(
        np.repeat(m2, 8, axis=1).astype(ml_dtypes.bfloat16))  # [128,16]
    w1t = np.ascontiguousarray(
        w1.T.reshape(2, 128, 16).transpose(1, 0, 2).reshape(128, 32))
    w2t = np.ascontiguousarray(w2.T)                     # [16,256]
    b1c = np.ascontiguousarray(b1.reshape(16, 1))
    b2c = np.ascontiguousarray(b2.reshape(2, 128).T)     # [128,2]

    key = (c1, c2)
    if key not in _prog_cache:
        _prog_cache[key] = _build_program(c1, c2)
    nc = _prog_cache[key]

    consts = {"mf": mf, "gm0": gms[1], "gm1": gms[2], "gm2": gms[3],
              "w1t": w1t, "w2t": w2t,
              "b1c": b1c, "b2c": b2c,
              "onr": np.ones((1, 128), dtype=ml_dtypes.bfloat16)}
    xh = x.reshape(B, C, HW).astype(ml_dtypes.bfloat16)
    in_maps = [{"xb": xh[i], **consts} for i in range(N_CORES)]

    from concourse.bass_utils import run_bass_kernel_spmd
    res = run_bass_kernel_spmd(nc, in_maps, list(range(N_CORES)),
                               trace=_TRACE)
    global _last_res
    _last_res = res
    out = np.stack([res.results[i]["out"].astype(np.float32).reshape(C, H, W)
                    for i in range(N_CORES)])
    return out
